# revision 1
# baseline (speedup 1.0000x reference)
"""Bass/Tile kernel for nn_MultiHeadAttention (B=2, S=2048, D=1024, H=16).

Sharding: 8 cores = 2 (batch) x 4 (head-chunks of 4 heads).
Each core computes, for its batch b and its 4 heads:
  qpT/kpT = (x @ W{q,k} + b)^T   in [dout, token] layout (2 pairs of 128)
  vp      = v @ Wv + bv          in [token, dout] layout
  scoresT = kp @ qp^T            per head, [k, q] layout
  attnT   = exp(scoresT)         (softmax over q == free axis; no max-sub needed)
  Z[k]    = sum_q attnT[k, q]    (ACT accum_out)
  outT    = sum_kb (vp[kb]/Z[kb])^T-style PV matmuls, [d, q] accumulated in PSUM
  partial = headcatT^T @ Wo_rows  -> [token, 1024] fp32 partial (host sums partials)

Layouts avoid all on-chip transposes: host ships qT/kT/vT ([D, S]) pre-transposed.
"""

import sys

sys.path.insert(0, "/opt/trn_rl_repo")

from contextlib import ExitStack

import numpy as np
import ml_dtypes

import concourse.bass as bass
import concourse.mybir as mybir
import concourse.tile as tile
from concourse import bacc
from concourse.bass_utils import run_bass_kernel_spmd

BF16 = mybir.dt.bfloat16
F32 = mybir.dt.float32
F32R = mybir.dt.float32r
AF = mybir.ActivationFunctionType
ALU = mybir.AluOpType

D = 1024
NK = 8  # k-tiles over D
DOUT = 256  # per-core head dims (4 heads)
NPAIR = 2  # pairs of heads (128 dout each)
HD = 64


def build_kernel(S=2048, use_tile_position=True):
    NKB = S // 128  # k-token blocks
    NQH = S // 1024  # exp blocks of 1024 along q
    NTC = S // 512  # proj token chunks
    NTT = S // 128  # token tiles
    assert S % 1024 == 0

    nc = bacc.Bacc("TRN2", target_bir_lowering=False, debug=False)

    qT = nc.dram_tensor("qT", [D, S], F32R, kind="ExternalInput")
    kT = nc.dram_tensor("kT", [D, S], F32R, kind="ExternalInput")
    vT = nc.dram_tensor("vT", [D, S], BF16, kind="ExternalInput")
    wq = nc.dram_tensor("wq", [D, DOUT], F32R, kind="ExternalInput")
    wk = nc.dram_tensor("wk", [D, DOUT], F32R, kind="ExternalInput")
    wv = nc.dram_tensor("wv", [D, DOUT], BF16, kind="ExternalInput")
    wo = nc.dram_tensor("wo", [DOUT, D], F32R, kind="ExternalInput")
    bq = nc.dram_tensor("bq", [NPAIR, 128, 1], F32, kind="ExternalInput")
    bk = nc.dram_tensor("bk", [NPAIR, 128, 1], F32, kind="ExternalInput")
    bv = nc.dram_tensor("bv", [DOUT], F32, kind="ExternalInput")
    out = nc.dram_tensor("out", [S, D], F32, kind="ExternalOutput")

    # tiled DRAM views
    qTv = qT.ap().rearrange("(t p) s -> t p s", p=128)  # [8, 128, S]
    kTv = kT.ap().rearrange("(t p) s -> t p s", p=128)
    vTv = vT.ap().rearrange("(t p) s -> t p s", p=128)
    wqv = wq.ap().rearrange("(t p) m -> p t m", p=128)  # [128, 8, 256]
    wkv = wk.ap().rearrange("(t p) m -> p t m", p=128)
    wvv = wv.ap().rearrange("(t p) m -> p t m", p=128)
    wov = wo.ap().rearrange("(t p) m -> p t m", p=128)  # [128, 2, 1024]
    bqv = bq.ap().rearrange("a p o -> p a o")  # [128, 2, 1]
    bkv = bk.ap().rearrange("a p o -> p a o")
    outv = out.ap().rearrange("(t p) m -> t p m", p=128)  # [NTT, 128, 1024]

    bv_bcast_ap = bass.AP(tensor=bv.ap().tensor, offset=0, ap=[[0, 128], [1, DOUT]])

    with tile.TileContext(nc) as tc, ExitStack() as ctx:
        sb = ctx.enter_context(tc.tile_pool(name="sb", bufs=1))

        # ---- resident loads ----
        wq_sb = sb.tile([128, NK, DOUT], F32R, tag="wq")
        wk_sb = sb.tile([128, NK, DOUT], F32R, tag="wk")
        wv_sb = sb.tile([128, NK, DOUT], BF16, tag="wv")
        wo_sb = sb.tile([128, NPAIR, D], F32R, tag="wo")
        nc.sync.dma_start(out=wq_sb[:], in_=wqv)
        nc.sync.dma_start(out=wk_sb[:], in_=wkv)
        bq_sb = sb.tile([128, NPAIR, 1], F32, tag="bq")
        bk_sb = sb.tile([128, NPAIR, 1], F32, tag="bk")
        bv_sb = sb.tile([128, DOUT], F32, tag="bv")
        nc.sync.dma_start(out=bq_sb[:], in_=bqv)
        nc.sync.dma_start(out=bk_sb[:], in_=bkv)
        nc.sync.dma_start(out=bv_sb[:], in_=bv_bcast_ap)

        vT_sb = sb.tile([128, NK, S], BF16, tag="vT")

        # ---- projection outputs (resident SBUF) ----
        qpT_sb = sb.tile([128, NPAIR, S], F32R, tag="qpT")
        kpT_sb = sb.tile([128, NPAIR, S], F32R, tag="kpT")
        vp_sb = sb.tile([128, NTT, DOUT], BF16, tag="vp")
        hcT_sb = sb.tile([128, NPAIR, S], F32R, tag="hcT")

        # fp32 PV accumulators in SBUF (DVE-accumulated); O-proj lhsT source
        hc_acc = sb.tile([128, NPAIR, S], F32, tag="hc_acc")

        # ---- single PSUM pool for the whole kernel: no pool-release
        # barriers between projections and attention.
        # banks: sc0(2) + sc1(2) + pvt(2) + aux(2) = 8
        asb0 = ctx.enter_context(tc.tile_pool(name="qk_stream", bufs=1))
        psa = ctx.enter_context(tc.tile_pool(name="ps_all", bufs=1, space="PSUM"))
        asb = ctx.enter_context(tc.tile_pool(name="att_sb", bufs=1))

        def emit_qkproj(XTv, W_sb, b_sb, XPT, tci):
            # one token-chunk of a q/k projection, both pairs, via aux slots
            tsl = slice(tci * 512, tci * 512 + 512)
            pss = []
            for p in range(NPAIR):
                ps_t = psa.tile([128, 512], F32, tag="aux", bufs=2, name=f"pj{p}")
                pss.append(ps_t)
            for kk in range(NK):
                xt = asb0.tile([128, 512], F32R, tag="xt", bufs=12, name="xt")
                nc.sync.dma_start(out=xt[:], in_=XTv[kk][:, tsl])
                for p in range(NPAIR):
                    nc.tensor.matmul(
                        pss[p][:],
                        lhsT=W_sb[:, kk, p * 128 : p * 128 + 128],
                        rhs=xt[:],
                        start=(kk == 0),
                        stop=(kk == NK - 1),
                    )
            for p in range(NPAIR):
                nc.vector.tensor_scalar_add(XPT[:, p, tsl], pss[p][:], b_sb[:, p, :])

        def emit_vproj(tt):
            psv = psa.tile([128, DOUT], F32, tag="aux", bufs=2, name="projv")
            for kk in range(NK):
                nc.tensor.matmul(
                    psv[:],
                    lhsT=vT_sb[:, kk, tt * 128 : tt * 128 + 128],
                    rhs=wv_sb[:, kk, :],
                    start=(kk == 0),
                    stop=(kk == NK - 1),
                )
            nc.vector.scalar_tensor_tensor(
                out=vp_sb[:, tt, :],
                in0=psv[:],
                scalar=1.0,
                in1=bv_sb[:],
                op0=ALU.mult,
                op1=ALU.add,
            )

        def emit_scores(p, kb):
            ksl = slice(kb * 128, kb * 128 + 128)
            scs = {}
            for qh in range(NQH):
                for h in range(2):
                    scs[(h, qh)] = psa.tile(
                        [128, 1024], F32, tag=f"sc{h}", bufs=1,
                        name=f"sc{h}{qh}",
                    )
                for qq in range(2):
                    for h in range(2):
                        hsl = slice(h * 64, h * 64 + 64)
                        qsl = slice(
                            qh * 1024 + qq * 512, qh * 1024 + qq * 512 + 512
                        )
                        nc.tensor.matmul(
                            scs[(h, qh)][:, qq * 512 : qq * 512 + 512],
                            lhsT=kpT_sb[hsl, p, ksl],
                            rhs=qpT_sb[hsl, p, qsl],
                            start=True,
                            stop=True,
                            tile_position=(h * 64, 0)
                            if use_tile_position == "scores_too"
                            else None,
                        )
            return scs

        def emit_exps(scs):
            at_tiles, z_parts = {}, {}
            for h in range(2):
                for qh in range(NQH):
                    at = asb.tile(
                        [128, 1024], BF16, tag=f"at{h}_{qh}", bufs=3,
                        name=f"at{h}{qh}",
                    )
                    z = asb.tile(
                        [128, 1], F32, tag=f"z{h}_{qh}", bufs=3, name=f"z{h}{qh}"
                    )
                    nc.scalar.activation(
                        out=at[:], in_=scs[(h, qh)][:], func=AF.Exp,
                        accum_out=z[:],
                    )
                    at_tiles[(h, qh)] = at
                    z_parts[(h, qh)] = z
            return at_tiles, z_parts

        def emit_pv(p, kb, at_tiles, z_parts):
            vhss = {}
            for h in range(2):
                if NQH > 1:
                    zs = asb.tile([128, 1], F32, tag=f"zs{h}", bufs=2, name="zs")
                    nc.vector.tensor_add(
                        zs[:], z_parts[(h, 0)][:], z_parts[(h, 1)][:]
                    )
                    for qh in range(2, NQH):
                        nc.vector.tensor_add(zs[:], zs[:], z_parts[(h, qh)][:])
                else:
                    zs = z_parts[(h, 0)]
                rz = asb.tile([128, 1], F32, tag=f"rz{h}", bufs=2, name="rz")
                nc.vector.reciprocal(rz[:], zs[:])
                vhs = asb.tile([128, HD], BF16, tag=f"vh{h}", bufs=2, name="vhs")
                nc.vector.tensor_scalar_mul(
                    vhs[:],
                    vp_sb[:, kb, p * 128 + h * 64 : p * 128 + h * 64 + 64],
                    rz[:],
                )
                vhss[h] = vhs
            for qh in range(NQH):
                pvt = psa.tile([128, 1024], F32, tag="pvt", bufs=1, name="pvt")
                for qq in range(2):
                    for h in range(2):
                        nc.tensor.matmul(
                            pvt[
                                h * 64 : h * 64 + 64,
                                qq * 512 : qq * 512 + 512,
                            ],
                            lhsT=vhss[h][:],
                            rhs=at_tiles[(h, qh)][
                                :, qq * 512 : qq * 512 + 512
                            ],
                            start=True,
                            stop=True,
                            tile_position=(0, h * 64)
                            if use_tile_position
                            else None,
                            skip_group_check=True,
                        )
                qsl = slice(qh * 1024, qh * 1024 + 1024)
                if kb == 0:
                    nc.vector.tensor_copy(hc_acc[:, p, qsl], pvt[:])
                else:
                    nc.vector.tensor_add(
                        hc_acc[:, p, qsl], hc_acc[:, p, qsl], pvt[:]
                    )


        osb = ctx.enter_context(tc.tile_pool(name="o_sb", bufs=1))

        # ---- head: q-proj (both pairs) + first k-chunk, then attention ----
        for tci in range(NTC):
            emit_qkproj(qTv, wq_sb, bq_sb, qpT_sb, tci)
        for tci in range(NTC):
            emit_qkproj(kTv, wk_sb, bk_sb, kpT_sb, tci)

        # deferred loads: not needed until PV / O-proj — keep them out of the
        # critical head DMA window
        nc.sync.dma_start(out=wv_sb[:], in_=wvv)
        for kk in range(NK):
            nc.sync.dma_start(out=vT_sb[:, kk, :], in_=vTv[kk])
        nc.sync.dma_start(out=wo_sb[:], in_=wov)

        # ---- attention, software-pipelined; remaining projections trickle
        # through the attention stream on the aux slots ----
        for p in range(NPAIR):
            scs = emit_scores(p, 0)
            if p == 0:
                emit_vproj(0)
            for kb in range(NKB):
                at_tiles, z_parts = emit_exps(scs)
                if kb + 1 < NKB:
                    scs = emit_scores(p, kb + 1)
                if p == 0 and kb + 1 < NKB:
                    emit_vproj(kb + 1)
                emit_pv(p, kb, at_tiles, z_parts)
            for cc in range(S // 512):
                if cc % 2 == 0:
                    nc.vector.tensor_copy(
                        hcT_sb[:, p, cc * 512 : cc * 512 + 512],
                        hc_acc[:, p, cc * 512 : cc * 512 + 512],
                    )
                else:
                    nc.scalar.copy(
                        hcT_sb[:, p, cc * 512 : cc * 512 + 512],
                        hc_acc[:, p, cc * 512 : cc * 512 + 512],
                    )

        # ---- O projection (aux slots) ----
        for tt in range(NTT):
            for dc in range(2):
                ps_t = psa.tile([128, 512], F32, tag="aux", bufs=2, name=f"o{dc}")
                for p in range(NPAIR):
                    nc.tensor.matmul(
                        ps_t[:],
                        lhsT=hcT_sb[:, p, tt * 128 : tt * 128 + 128],
                        rhs=wo_sb[:, p, dc * 512 : dc * 512 + 512],
                        start=(p == 0),
                        stop=(p == NPAIR - 1),
                    )
                ost = osb.tile(
                    [128, 512], F32, tag=f"ost{dc}", bufs=2, name=f"ost{dc}"
                )
                if (tt + dc) % 2 == 0:
                    nc.vector.tensor_copy(ost[:], ps_t[:])
                else:
                    nc.scalar.copy(ost[:], ps_t[:])
                nc.sync.dma_start(
                    out=outv[tt][:, dc * 512 : dc * 512 + 512], in_=ost[:]
                )

    nc.compile()
    return nc


# ---------------- host-side shard / unshard ----------------

S = 2048
B = 2

_NC_CACHE = {}


def _get_nc():
    if "nc" not in _NC_CACHE:
        _NC_CACHE["nc"] = build_kernel(S=S)
    return _NC_CACHE["nc"]


def make_in_maps(q, k, v, Wq, bq, Wk, bk, Wv, bv, Wo, bo):
    bf = ml_dtypes.bfloat16
    maps = []
    for c in range(8):
        b = c // 4
        hc = c % 4
        cols = slice(256 * hc, 256 * hc + 256)
        maps.append({
            "qT": np.ascontiguousarray(q[b].T.astype(np.float32)),
            "kT": np.ascontiguousarray(k[b].T.astype(np.float32)),
            "vT": np.ascontiguousarray(v[b].astype(bf).T),
            "wq": np.ascontiguousarray(Wq[:, cols].astype(np.float32)),
            "wk": np.ascontiguousarray(Wk[:, cols].astype(np.float32)),
            "wv": np.ascontiguousarray(Wv[:, cols].astype(bf)),
            "wo": np.ascontiguousarray(Wo[cols, :].astype(np.float32)),
            "bq": np.ascontiguousarray(
                bq[cols].reshape(NPAIR, 128, 1).astype(np.float32)
            ),
            "bk": np.ascontiguousarray(
                bk[cols].reshape(NPAIR, 128, 1).astype(np.float32)
            ),
            "bv": np.ascontiguousarray(bv[cols].astype(np.float32)),
        })
    return maps


def kernel(q, k, v, Wq, bq, Wk, bk, Wv, bv, Wo, bo):
    q = np.asarray(q, dtype=np.float32)
    k = np.asarray(k, dtype=np.float32)
    v = np.asarray(v, dtype=np.float32)
    Wq = np.asarray(Wq, dtype=np.float32)
    Wk = np.asarray(Wk, dtype=np.float32)
    Wv = np.asarray(Wv, dtype=np.float32)
    Wo = np.asarray(Wo, dtype=np.float32)
    bq = np.asarray(bq, dtype=np.float32)
    bk = np.asarray(bk, dtype=np.float32)
    bv = np.asarray(bv, dtype=np.float32)
    bo = np.asarray(bo, dtype=np.float32)

    nc = _get_nc()
    maps = make_in_maps(q, k, v, Wq, bq, Wk, bk, Wv, bv, Wo, bo)
    res = run_bass_kernel_spmd(nc, maps, core_ids=list(range(8)))

    outs = []
    for b in range(B):
        acc = np.zeros((S, D), dtype=np.float32)
        for hc in range(4):
            acc += res.results[b * 4 + hc]["out"]
        acc += bo[None, :]
        outs.append(acc)
    return np.stack(outs, axis=0)

